# revision 11
# baseline (speedup 1.0000x reference)
"""DSA varlen sparse attention for Trainium2, 8 NeuronCores.

Strategy (token-sharded, K/V replicated per core):
  Per core c: tokens t in [c*256, (c+1)*256).
  Dense per-head scores S^T[j, t] = sum_d K[j,h,d] q[t,h,d] on the PE
  array in bf16; softmax's Z cancels in the reference's renormalization,
  so the output is exactly
     out[t,h] = (sum_j exp(s[j,t]) * tsd[j,t] * V[j,h]) / (sum_j exp*tsd)
  where tsd[j,t] = sum_{k: topk_idx[t,k]=j} topk_scores[t,k].

  tsd is built HOST-side (pure reformatting of topk_indices/topk_scores)
  and DMA'd in dense [j, t] layout.  Device pipeline per head:
     PE:  16 S^T chunk-matmuls [128j x 256t] -> PSUM, then 16x2 AV
          matmuls [128t x 129] accumulating over j-chunks
     ACT: exp(scale*S^T) PSUM->SBUF bf16 (structural bottleneck ~32us)
     DVE: mask-mult by tsdT (2x mode), reciprocal + normalize
  K/q stream per-head ahead of ACT on the single sync HWDGE queue
  (FIFO = priority order); V/tsdT interleave behind.  Head 7 runs
  t-split so its mask/AV/store overlap the final exps (short tail).
  The denominator rides as a leading "ones" column of V.
"""

import numpy as np
import ml_dtypes
from contextlib import ExitStack

T, H, D, DV, TK = 2048, 8, 128, 128, 64
NCORES = 8
TC = T // NCORES          # 256 tokens per core
P = 128
TCH = TC // P             # 2 token chunks of 128
JC = T // P               # 16 key chunks of 128
SCALE = float(D) ** -0.5

_CACHE = {}


def _build_program():
    import concourse.mybir as mybir
    import concourse.tile as tile
    from concourse import bacc

    dt = mybir.dt
    Alu = mybir.AluOpType
    Act = mybir.ActivationFunctionType

    nc = bacc.Bacc(None, target_bir_lowering=False, debug=False)
    names = {}
    with ExitStack() as ctx:
        tc = ctx.enter_context(tile.TileContext(nc))
        dram = ctx.enter_context(tc.tile_pool(name="dram", bufs=1, space="DRAM"))
        sb = ctx.enter_context(tc.tile_pool(name="sb", bufs=1))
        pT_pool = ctx.enter_context(tc.tile_pool(name="pTp", bufs=4))
        sps = ctx.enter_context(tc.tile_pool(name="spsum", bufs=2, space="PSUM"))
        ops = ctx.enter_context(tc.tile_pool(name="opsum", bufs=2, space="PSUM"))

        # ---------------- DRAM I/O (bf16 data prepped host-side) ----------
        q_d = dram.tile([P, H * TC], dt.bfloat16, kind="ExternalInput")
        k_d = dram.tile([P, H * T], dt.bfloat16, kind="ExternalInput")
        v_d = dram.tile([P, H * JC * (1 + DV)], dt.bfloat16, kind="ExternalInput")
        m_d = dram.tile([P, JC * TC], dt.bfloat16, kind="ExternalInput")
        out_d = dram.tile([P, TCH * H * DV], dt.float32, kind="ExternalOutput")

        names.update(q=q_d.name, k=k_d.name, v=v_d.name, m=m_d.name, out=out_d.name)

        # ---------------- SBUF persistent ----------------
        kT = sb.tile([P, H, T], dt.bfloat16, tag="kT")                 # 32KB/p
        vE = sb.tile([P, H, JC, 1 + DV], dt.bfloat16, tag="vE")        # 33KB/p
        qT = sb.tile([P, H, TC], dt.bfloat16, tag="qT")                # 4KB/p
        tsdT = sb.tile([P, JC, TC], dt.bfloat16, tag="tsdT")           # 8KB/p
        outs = sb.tile([P, TCH, H * DV], dt.float32, tag="outs")       # 8KB/p
        scratch = sb.tile([P, TC], dt.bfloat16, tag="scratch")         # warmup

        # ---------------- loads (single HWDGE queue, FIFO = priority) -----
        HVB = JC * (1 + DV)               # one head's V block

        def ld_k(h, c0, c1):
            nc.sync.dma_start(
                out=kT[:, h, c0:c1], in_=k_d[:, h * T + c0 : h * T + c1]
            )

        def ld_v(h0, h1):
            nc.sync.dma_start(
                out=vE[:, h0:h1].rearrange("p a b c -> p (a b c)"),
                in_=v_d[:, h0 * HVB : h1 * HVB],
            )

        def ld_q(h):
            nc.sync.dma_start(
                out=qT[:, h, :], in_=q_d[:, h * TC : (h + 1) * TC]
            )

        ld_k(0, 0, 512)
        ld_q(0)
        ld_k(0, 512, 1280)
        ld_k(0, 1280, 2048)
        ld_k(1, 0, 2048)
        ld_q(1)
        nc.sync.dma_start(out=tsdT[:].rearrange("p a b -> p (a b)"), in_=m_d[:])
        ld_k(2, 0, 2048)
        ld_q(2)
        ld_v(0, 1)
        ld_k(3, 0, 2048)
        ld_q(3)
        ld_v(1, 2)
        ld_k(4, 0, 2048)
        ld_q(4)
        ld_v(2, 3)
        ld_k(5, 0, 2048)
        ld_q(5)
        ld_v(3, 4)
        ld_k(6, 0, 2048)
        ld_q(6)
        ld_v(4, 5)
        ld_k(7, 0, 2048)
        ld_q(7)
        ld_v(5, 8)

        # ------- pipelined per-head phases --------------------------------
        # Heads 0-6: S^T in groups of (2,4,6,4)/(6,6,4) chunks per PSUM
        # tile; one exp instr per group.  Head 7: t-split groups of 8
        # chunk-halves so mask/AV overlap the last exps.
        pTs = [
            pT_pool.tile([P, JC, TC], dt.bfloat16, tag="pT", name=f"pT{i}")
            for i in range(4)
        ]

        def pT_of(h):
            return pTs[h % 4]

        def emit_st_group(h, jc0, njc):
            """One PSUM tile: njc chunk-matmuls + one exp -> pT rows."""
            sp = sps.tile([P, 6, TC], dt.float32, tag="sp")
            pT = pT_of(h)
            for j in range(njc):
                jc = jc0 + j
                nc.tensor.matmul(
                    out=sp[:, j, :],
                    lhsT=kT[:, h, jc * P : (jc + 1) * P],
                    rhs=qT[:, h, :],
                    start=True, stop=True,
                )
            nc.scalar.activation(
                out=pT[:, jc0 : jc0 + njc, :], in_=sp[:, 0:njc, :],
                func=Act.Exp, scale=SCALE,
            )

        def emit_st7_group(g):
            """Head 7, t-split: group g covers t-half g//2, chunks 8*(g%2)."""
            t, half = g // 2, g % 2
            sp = sps.tile([P, 6, TC], dt.float32, tag="sp")
            spv = sp.rearrange("p a (b c) -> p (a b) c", c=P)
            pT = pT_of(7)
            for j in range(8):
                jc = half * 8 + j
                nc.tensor.matmul(
                    out=spv[:, j, :],
                    lhsT=kT[:, 7, jc * P : (jc + 1) * P],
                    rhs=qT[:, 7, t * P : (t + 1) * P],
                    start=True, stop=True,
                )
            nc.scalar.activation(
                out=pT[:, half * 8 : half * 8 + 8, t * P : (t + 1) * P],
                in_=spv[:, 0:8, :],
                func=Act.Exp, scale=SCALE,
            )

        def emit_mask(h, t=None):
            pT = pT_of(h)
            sl = slice(None) if t is None else slice(t * P, (t + 1) * P)
            nc.vector.tensor_tensor(
                out=pT[:, :, sl], in0=pT[:, :, sl], in1=tsdT[:, :, sl],
                op=Alu.mult,
            )

        def emit_av(h, t):
            pT = pT_of(h)
            op = ops.tile([P, 1 + DV], dt.float32, tag="op")
            for jc in range(JC):
                nc.tensor.matmul(
                    out=op[:],
                    lhsT=pT[:, jc, t * P : (t + 1) * P],
                    rhs=vE[:, h, jc, :],
                    start=(jc == 0), stop=(jc == JC - 1),
                )
            avps[(h, t)] = op

        def emit_norm(h, t):
            op = avps[(h, t)]
            rec = sb.tile([P, 1], dt.float32, tag=f"rec{h}_{t}")
            nc.vector.reciprocal(out=rec[:], in_=op[:, 0:1])
            dst = outs[:, t, h * DV : (h + 1) * DV]
            nc.vector.tensor_scalar(
                out=dst, in0=op[:, 1 : 1 + DV],
                scalar1=rec[:], scalar2=None, op0=Alu.mult,
            )
            nc.sync.dma_start(
                out=out_d[:, (t * H + h) * DV : (t * H + h + 1) * DV], in_=dst
            )

        avps = {}
        GROUPS = {0: (4, 6, 6)}
        for h in range(1, 7):
            GROUPS[h] = (6, 6, 4)

        # PE warmup: dummy matmuls on never-written scratch start the HAM
        # clock ramp during the DMA fill (no data deps, garbage results
        # into a PSUM tile nothing reads).
        nc.vector.memset(scratch[:], 0.0)
        wp = sps.tile([P, 6, TC], dt.float32, tag="sp")
        for i in range(13):
            nc.tensor.matmul(
                out=wp[:, 0, :], lhsT=scratch[:, 0:P],
                rhs=scratch[:, 0:TC], start=True, stop=True,
            )

        def emit_st_head(h):
            jc0 = 0
            for njc in GROUPS[h]:
                emit_st_group(h, jc0, njc)
                jc0 += njc

        # PE/ACT stream with AVs interleaved at group granularity;
        # DVE order: masks prioritized, norm(h) after mask(h+2).
        emit_st_head(0)
        emit_st_head(1)
        # h=2..6: st(h) groups; av(h-2,0) after g1, av(h-2,1) after g2
        for h in range(2, 7):
            g = GROUPS[h]
            emit_st_group(h, 0, g[0])
            if h == 2:
                emit_mask(0)
                emit_mask(1)
            emit_st_group(h, g[0], g[1])
            emit_av(h - 2, 0)
            emit_st_group(h, g[0] + g[1], g[2])
            emit_av(h - 2, 1)
            emit_mask(h)
            if h >= 4:
                emit_norm(h - 4, 0)
                emit_norm(h - 4, 1)
        # st7 t-split groups; avs for h5/h6 slotted between
        emit_st7_group(0)
        emit_st7_group(1)
        emit_av(5, 0)
        emit_st7_group(2)
        emit_av(5, 1)
        emit_norm(3, 0)
        emit_norm(3, 1)
        emit_st7_group(3)
        emit_av(6, 0)
        emit_mask(7, 0)
        emit_norm(4, 0)
        emit_norm(4, 1)
        emit_av(6, 1)
        emit_av(7, 0)
        emit_mask(7, 1)
        emit_norm(5, 0)
        emit_norm(5, 1)
        emit_av(7, 1)
        emit_norm(6, 0)
        emit_norm(6, 1)
        emit_norm(7, 0)
        emit_norm(7, 1)

    nc.compile()
    return nc, names


def _get_program():
    key = "prog"
    if key not in _CACHE:
        _CACHE[key] = _build_program()
    return _CACHE[key]


def _host_inputs(q, k, v, idx, ts):
    """Build per-core in_maps (host-side shard/layout/dtype prep)."""
    bf16 = ml_dtypes.bfloat16

    # kT[d, h, j] = K[j, h, d]  (device reads it as [P, H*T])
    k_full = np.ascontiguousarray(
        k.transpose(2, 1, 0).reshape(P, H * T)
    ).astype(bf16)
    # vE[p, h, jc, 0] = 1, vE[p, h, jc, 1:] = V[jc*128+p, h, :]
    v_r = v.reshape(JC, P, H, DV).transpose(1, 2, 0, 3)  # [P, H, JC, DV]
    v_full = np.ones((P, H, JC, 1 + DV), dtype=np.float32)
    v_full[:, :, :, 1:] = v_r
    v_full = v_full.reshape(P, H * JC * (1 + DV)).astype(bf16)

    # Dense mask W[t, j] = sum_{k: idx[t,k]=j} ts[t,k]  (host scatter-add)
    flat = (np.arange(T, dtype=np.int64)[:, None] * T + idx).ravel()
    W = np.bincount(flat, weights=ts.astype(np.float64).ravel(), minlength=T * T)
    W = W.reshape(T, T).astype(np.float32)

    maps = []
    for c in range(NCORES):
        sl = slice(c * TC, (c + 1) * TC)
        # qT[d, h, t] with t local to the shard
        qc = q[sl].transpose(2, 1, 0).reshape(P, H * TC)
        # tsdT[p, jc, t] = W[t_global, jc*128 + p]
        mc = W[sl].reshape(TC, JC, P).transpose(2, 1, 0).reshape(P, JC * TC)
        maps.append(
            dict(
                q=np.ascontiguousarray(qc).astype(bf16),
                k=k_full,
                v=v_full,
                m=np.ascontiguousarray(mc).astype(bf16),
            )
        )
    return maps


def kernel(q_packed, k_packed, v_packed, topk_indices, topk_scores):
    from concourse.bass_utils import run_bass_kernel_spmd

    q = np.asarray(q_packed, dtype=np.float32)
    k = np.asarray(k_packed, dtype=np.float32)
    v = np.asarray(v_packed, dtype=np.float32)
    idx = np.asarray(topk_indices)
    ts = np.asarray(topk_scores, dtype=np.float32)

    nc, names = _get_program()
    logical_maps = _host_inputs(q, k, v, idx, ts)
    in_maps = [{names[key]: arr for key, arr in m.items()} for m in logical_maps]

    res = run_bass_kernel_spmd(nc, in_maps, core_ids=list(range(NCORES)))
    outn = names["out"]
    parts = []
    for c in range(NCORES):
        oc = res.results[c][outn].reshape(P, TCH, H, DV)  # [p, t, h, dv]
        parts.append(oc.transpose(1, 0, 2, 3).reshape(TC, H, DV))
    return np.concatenate(parts, axis=0).astype(np.float32)


if __name__ == "__main__":
    rng = np.random.default_rng(0)
    q = rng.standard_normal((T, H, D), dtype=np.float32)
    k = rng.standard_normal((T, H, D), dtype=np.float32)
    v = rng.standard_normal((T, H, DV), dtype=np.float32)
    idx = rng.integers(0, T, size=(T, TK), dtype=np.int64)
    ts = rng.random((T, TK), dtype=np.float32)
    out = kernel(q, k, v, idx, ts)
    print(out.shape, out.dtype)
